# revision 17
# baseline (speedup 1.0000x reference)
"""AdditiveAttention on 8 TRN2 NeuronCores — data-parallel over batch.

Per core (one batch element b):
  qh[h,q] = sum_d Wq[d,h] * queries[b,q,d]           (TensorE, H on partitions)
  kh[h,k] = sum_d Wk[d,h] * keys[b,k,d]              (TensorE)
  for each q: feat[h,k] = tanh(kh[h,k] + qh[h,q])    (ScalarE, bias-fused add)
              scT[k,q] += feat[h,k-blk].T @ wv[h]     (TensorE, feat stationary,
                                                       wv moving N=1 -> base part 0)
  masked softmax over k, no max-subtract (|scores|<~4): exp fuses the mask via
  scale/bias (valid -> exp(score); masked -> exp(-1e6)=0; vlen==0 -> uniform)
  out[q,v] = sum_k pT[k,q] * values[b,k,v]           (TensorE, pT is already lhsT)

bf16 is used for all large matmuls (full PE rate, half DMA bytes); the
tanh/softmax accumulation paths stay fp32.
Host side only reshapes/shards inputs and stacks the 8 per-core outputs.
"""

import ml_dtypes
import numpy as np

B, LQ, LK, D, H, DV = 8, 128, 1024, 512, 256, 512
NEG = -1000000.0
NCORES = 8


def _build_program():
    import concourse.mybir as mybir
    import concourse.tile as tile
    from concourse import bacc

    f32 = mybir.dt.float32
    f32r = mybir.dt.float32r
    bf16 = mybir.dt.bfloat16
    AF = mybir.ActivationFunctionType

    nc = bacc.Bacc(
        "TRN2",
        target_bir_lowering=False,
        debug=False,
        num_devices=NCORES,
    )

    # Per-core DRAM parameters (host passes transposed layouts).
    qT_ext = nc.dram_tensor("qT", [D, LQ], bf16, kind="ExternalInput").ap()
    kT_ext = nc.dram_tensor("kT", [D, LK], bf16, kind="ExternalInput").ap()
    val_ext = nc.dram_tensor("values", [LK, DV], bf16, kind="ExternalInput").ap()
    wq_ext = nc.dram_tensor("Wq", [D, H], bf16, kind="ExternalInput").ap()
    wk_ext = nc.dram_tensor("Wk", [D, H], bf16, kind="ExternalInput").ap()
    wv_ext = nc.dram_tensor("wv2", [128, 2], f32, kind="ExternalInput").ap()
    mcol_ext = nc.dram_tensor("mcol", [128, 8], f32, kind="ExternalInput").ap()
    ncol_ext = nc.dram_tensor("ncol", [128, 8], f32, kind="ExternalInput").ap()
    out_ext = nc.dram_tensor("out", [LQ, DV], f32, kind="ExternalOutput").ap()

    DC = D // 128   # 4 contraction chunks
    HC = H // 128   # 2 h chunks
    KC = LK // 128  # 8 key chunks
    KH = LK // 512  # 2 key halves (psum bank width)

    with tile.TileContext(nc) as tc:
        with (
            tc.tile_pool(name="const", bufs=1) as const,
            tc.tile_pool(name="feat", bufs=6) as featp,
            tc.tile_pool(name="pscore", bufs=1, space="PSUM") as pscore,
            tc.tile_pool(name="ptmp", bufs=2, space="PSUM") as ptmp,
            tc.tile_pool(name="psmall", bufs=1, space="PSUM") as psmall,
            tc.tile_pool(name="pout", bufs=1, space="PSUM") as pout,
        ):
            # ---- resident SBUF tiles -------------------------------------
            qsT = const.tile([128, DC, LQ], bf16, tag="qsT")      # queries^T
            ksT = const.tile([128, DC, LK], bf16, tag="ksT")      # keys^T
            wq_sb = const.tile([128, DC, H], bf16, tag="wq")
            wk_sb = const.tile([128, DC, H], bf16, tag="wk")
            wv_sb = const.tile([128, 2], f32, tag="wv")
            mcol = const.tile([128, 8], f32, tag="mcol")
            ncol = const.tile([128, 8], f32, tag="ncol")
            vals = const.tile([128, KC, DV], bf16, tag="vals")
            ones = const.tile([128, 1], bf16, tag="ones")
            qh_sb = const.tile([128, H], f32, tag="qh")           # qh[h, q] chunks
            kh_sb = const.tile([128, HC * LK], f32, tag="kh")     # kh[h, k] chunks
            pT_sb = const.tile([128, KC * LQ], bf16, tag="pT")     # exp(scT) chunks
            rinv = const.tile([LQ, 1], f32, tag="rinv")
            out_sb = const.tile([LQ, DV], f32, tag="outsb")
            warm = const.tile([128, 512], bf16, tag="warm")
            scratch = const.tile([1, 1], f32, tag="scratch")

            nc.vector.memset(ones[:], 1.0)
            nc.vector.memset(warm[:], 0.0)

            # ---- DMA loads: one consolidated DMA per tensor, all on the
            # sync HWDGE ring (keeps the ACT sequencer free), ordered by
            # when the pipeline needs them. values is emitted mid-loop so
            # its 2MB doesn't compete with kT on the DMA engines.
            nc.sync.dma_start(
                ksT[:, :, 0:512],
                kT_ext[:, 0:512].rearrange("(c p) k -> p c k", p=128),
            )
            nc.sync.dma_start(
                wk_sb[:, :, 0:128],
                wk_ext[:, 0:128].rearrange("(c p) h -> p c h", p=128),
            )
            nc.sync.dma_start(
                ksT[:, :, 512:1024],
                kT_ext[:, 512:1024].rearrange("(c p) k -> p c k", p=128),
            )
            nc.sync.dma_start(
                qsT[:, :, :], qT_ext.rearrange("(c p) q -> p c q", p=128)
            )
            nc.sync.dma_start(
                wq_sb[:, :, 0:128],
                wq_ext[:, 0:128].rearrange("(c p) h -> p c h", p=128),
            )
            nc.sync.dma_start(
                wk_sb[:, :, 128:256],
                wk_ext[:, 128:256].rearrange("(c p) h -> p c h", p=128),
            )
            nc.sync.dma_start(
                wq_sb[:, :, 128:256],
                wq_ext[:, 128:256].rearrange("(c p) h -> p c h", p=128),
            )
            nc.sync.dma_start(wv_sb[:], wv_ext[:])
            nc.sync.dma_start(mcol[:], mcol_ext[:])
            nc.sync.dma_start(ncol[:], ncol_ext[:])

            # ---- PE warmup: keep the tensor engine continuously busy on a
            # zero tile while kT streams in, so the projections (and the
            # loop) run at full clock instead of the mid power-state.
            for w in range(8):
                wt = ptmp.tile([128, 512], f32, tag="ptmp")
                nc.tensor.matmul(
                    wt[:], lhsT=warm[:, 0:128], rhs=warm[:], start=True, stop=True
                )

            # ---- projections: kh[h,k] (hc=0 first), qh[h,q] -------------
            def project_k(hc):
                for kh in range(KH):
                    pt = ptmp.tile([128, 512], f32, tag="ptmp")
                    for dc in range(DC):
                        nc.tensor.matmul(
                            pt[:],
                            lhsT=wk_sb[:, dc, hc * 128:(hc + 1) * 128],
                            rhs=ksT[:, dc, kh * 512:(kh + 1) * 512],
                            start=(dc == 0),
                            stop=(dc == DC - 1),
                        )
                    nc.vector.tensor_copy(
                        kh_sb[:, hc * LK + kh * 512: hc * LK + (kh + 1) * 512], pt[:]
                    )

            def project_q(hc):
                pt = ptmp.tile([128, 512], f32, tag="ptmp")
                for dc in range(DC):
                    nc.tensor.matmul(
                        pt[:, 0:LQ],
                        lhsT=wq_sb[:, dc, hc * 128:(hc + 1) * 128],
                        rhs=qsT[:, dc, :],
                        start=(dc == 0),
                        stop=(dc == DC - 1),
                    )
                nc.vector.tensor_copy(qh_sb[:, hc * LQ:(hc + 1) * LQ], pt[:, 0:LQ])

            project_k(0)
            project_q(0)
            project_k(1)
            project_q(1)

            # ---- main loop: tanh + wv-reduction into scores^T -----------
            # Tail work is staggered through the loop so the in-order ACT
            # stream never stalls on PE results.
            scT = pscore.tile([128, KC, LQ], f32, tag="scT")  # [k-part, kc, q]
            ssum = psmall.tile([LQ, 1], f32, tag="ssum")
            po = pout.tile([LQ, DV], f32, tag="po")
            QN = LQ // 2

            def tail_exps(qh0):
                # exp(mcol*score + ncol): valid -> exp(score), masked ->
                # exp(-1e6) = 0; vlen==0 -> exp(0) = 1 everywhere (uniform).
                for kc in range(KC):
                    nc.scalar.activation(
                        pT_sb[:, kc * LQ + qh0: kc * LQ + qh0 + QN],
                        scT[:, kc, qh0:qh0 + QN],
                        AF.Exp,
                        bias=ncol[:, kc:kc + 1],
                        scale=mcol[:, kc:kc + 1],
                    )
                for kc in range(KC):
                    nc.tensor.matmul(
                        ssum[qh0:qh0 + QN, :],
                        lhsT=pT_sb[:, kc * LQ + qh0: kc * LQ + qh0 + QN],
                        rhs=ones[:],
                        start=(kc == 0),
                        stop=(kc == KC - 1),
                    )
                nc.vector.reciprocal(rinv[qh0:qh0 + QN, :], ssum[qh0:qh0 + QN, :])

            def tail_exps_attnv(qh0):
                # end-tail variant: interleave sum and attn@values matmuls
                # per kc so PE advances as each exp lands.
                for kc in range(KC):
                    nc.scalar.activation(
                        pT_sb[:, kc * LQ + qh0: kc * LQ + qh0 + QN],
                        scT[:, kc, qh0:qh0 + QN],
                        AF.Exp,
                        bias=ncol[:, kc:kc + 1],
                        scale=mcol[:, kc:kc + 1],
                    )
                for kc in range(KC):
                    nc.tensor.matmul(
                        ssum[qh0:qh0 + QN, :],
                        lhsT=pT_sb[:, kc * LQ + qh0: kc * LQ + qh0 + QN],
                        rhs=ones[:],
                        start=(kc == 0),
                        stop=(kc == KC - 1),
                        skip_group_check=True,
                    )
                    nc.tensor.matmul(
                        po[qh0:qh0 + QN, :],
                        lhsT=pT_sb[:, kc * LQ + qh0: kc * LQ + qh0 + QN],
                        rhs=vals[:, kc, :],
                        start=(kc == 0),
                        stop=(kc == KC - 1),
                        skip_group_check=True,
                    )
                nc.vector.reciprocal(rinv[qh0:qh0 + QN, :], ssum[qh0:qh0 + QN, :])

            def tail_attnv(qh0):
                for kc in range(KC):
                    nc.tensor.matmul(
                        po[qh0:qh0 + QN, :],
                        lhsT=pT_sb[:, kc * LQ + qh0: kc * LQ + qh0 + QN],
                        rhs=vals[:, kc, :],
                        start=(kc == 0),
                        stop=(kc == KC - 1),
                    )

            def tail_out(qh0):
                # normalize rows by 1/sumexp during PSUM->SBUF copy
                nc.scalar.activation(
                    out_sb[qh0:qh0 + QN, :], po[qh0:qh0 + QN, :],
                    AF.Copy, bias=0.0, scale=rinv[qh0:qh0 + QN, :],
                )
                nc.sync.dma_start(
                    out_ext[qh0:qh0 + QN, :], out_sb[qh0:qh0 + QN, :]
                )

            for q in range(LQ):
                feats = []
                for hc in range(HC):
                    feat = featp.tile([128, LK], f32, tag="feat")
                    nc.scalar.activation(
                        feat[:],
                        kh_sb[:, hc * LK:(hc + 1) * LK],
                        AF.Tanh,
                        bias=qh_sb[:, hc * LQ + q: hc * LQ + q + 1],
                        scale=1.0,
                    )
                    feats.append(feat)
                for kc in range(KC):
                    for hc in range(HC):
                        nc.tensor.matmul(
                            scT[:, kc, q:q + 1],
                            lhsT=feats[hc][:, kc * 128:(kc + 1) * 128],
                            rhs=wv_sb[:, hc:hc + 1],
                            start=(hc == 0),
                            stop=(hc == HC - 1),
                        )
                if q == 4:
                    # values are only needed from ~q70. The copy below writes
                    # into the vals tile with a read of kh_sb, so the DMA
                    # (same-tile WAW) cannot be hoisted ahead of the
                    # prologue's own transfers on the shared DMA engines.
                    nc.gpsimd.tensor_copy(vals[0:1, 0, 0:1], kh_sb[0:1, 0:1])
                    nc.gpsimd.dma_start(
                        vals[:, :, :],
                        val_ext.rearrange("(c p) v -> p c v", p=128),
                    )
                elif q == QN + 3:
                    tail_exps(0)
                elif q == QN + 10:
                    tail_attnv(0)
                elif q == QN + 18:
                    tail_out(0)
            tail_exps_attnv(QN)
            tail_out(QN)

    nc.compile()
    return nc


def _make_in_maps(queries, keys, values, Wq, Wk, wv, valid_lens):
    queries = np.asarray(queries, dtype=np.float32)
    keys = np.asarray(keys, dtype=np.float32)
    values = np.asarray(values, dtype=np.float32)
    Wq = np.ascontiguousarray(np.asarray(Wq, dtype=np.float32))
    Wk = np.ascontiguousarray(np.asarray(Wk, dtype=np.float32))
    wv = np.asarray(wv, dtype=np.float32)
    vlens = np.asarray(valid_lens)

    Wq_bf = Wq.astype(ml_dtypes.bfloat16)
    Wk_bf = Wk.astype(ml_dtypes.bfloat16)
    wv2 = np.ascontiguousarray(wv.reshape(2, 128).T)
    karange = np.arange(LK).reshape(8, 128).T  # [p, kc] -> k index
    in_maps = []
    for c in range(NCORES):
        vlen = int(vlens[c])
        if vlen == 0:
            mcol = np.zeros((128, 8), dtype=np.float32)
            ncol = np.zeros((128, 8), dtype=np.float32)
        else:
            valid = karange < vlen
            mcol = valid.astype(np.float32)
            ncol = np.where(valid, 0.0, NEG).astype(np.float32)
        in_maps.append(
            {
                "qT": np.ascontiguousarray(queries[c].T).astype(ml_dtypes.bfloat16),
                "kT": np.ascontiguousarray(keys[c].T).astype(ml_dtypes.bfloat16),
                "values": np.ascontiguousarray(values[c]).astype(ml_dtypes.bfloat16),
                "Wq": Wq_bf,
                "Wk": Wk_bf,
                "wv2": wv2,
                "mcol": mcol,
                "ncol": ncol,
            }
        )
    return in_maps


def kernel(queries, keys, values, Wq, Wk, wv, valid_lens):
    from concourse.bass_utils import run_bass_kernel_spmd

    nc = _build_program()
    in_maps = _make_in_maps(queries, keys, values, Wq, Wk, wv, valid_lens)
    res = run_bass_kernel_spmd(nc, in_maps, core_ids=list(range(NCORES)))
    out = np.stack([res.results[c]["out"] for c in range(NCORES)], axis=0)
    return out
